# revision 17
# baseline (speedup 1.0000x reference)
"""Trainium2 Bass kernel for nn_Criterion (retrieval_knn, B=4 V=8192 F=16384 N=8192).

Per-core work (8 cores, data-parallel over B with 2-way split of N per batch):
  phase 0: gather face vertices (indirect DMA from DRAM), compute centers /
           unit normals / plane offsets; store the negated-score rhs table
           caug[24,F] (bf16 hi/mid/lo split) and gather table T[F,4] in DRAM.
  phase 1: per 128-point chunk, per 2048-face block: PE matmul (score' =
           2p.c - |c|^2 = -score) into PSUM; fused DVE prefix-MAX scan with
           carry-in scalar + accum carry-out; ACT Sign pass counts elements
           strictly below the block carry. A tiny fused DVE "pick" op turns
           the 8 (carry, count) pairs into the exact first-occurrence argmax.
  finish:  indirect-gather (n, c.n) per point, distance = p.n - c.n,
           interp = relu(eps - d); outputs per-partition partial sums.
Host sums the 8x128x2 partials into (loss, perc).
"""

import numpy as np

B, V, F, N = 4, 8192, 16384, 8192
NCORES = 8
SPLIT = 2                 # cores per batch
NS = N // SPLIT           # 4096 points per core
P = 128
CHUNKS = NS // P          # 32
FT = 2048                 # faces per scan block (4 PSUM banks)
NBLK = F // FT            # 8
FP = F // P               # 128 faces per partition in phase 0
EPS = 1e-3
WEIGHT = 1000.0
BIG = 3.0e38

_CACHE = {}


def _register_dve_op(name, spec):
    from concourse import dve_ops as D
    from concourse.dve_spec import lower

    for op in D.OPS:
        if op.name == name:
            return op
    op = D.DveOp(name, spec, subdim=False, uops_sha={})
    D.OPS.append(op)
    D._SUB_OPCODE_FOR_NAME[op.name] = D._CUSTOM_DVE_ROW_BASE + len(D.OPS) - 1
    D.CUSTOM_DVE_SPECS[op.name] = spec
    for ver in ("v3", "v4"):
        s = D.DveOpSpec(name=op.name, opcode=D.get_dve_sub_opcode(op.name),
                        uops=lower(spec, ver=ver), rd1_en=D.has_src1(spec))
        op.uops_sha[ver] = s.sha(ver)
    return op


def _get_ops():
    """Two fused custom-DVE ops (single-stream, ~1 elem/cycle fp32):

    maxscan: out[j] = max(C0, in0[0..j]) inclusive prefix-max with carry-in
             scalar C0; accum_out = max over the stream (= carry-out).
    pick:    among blocks t whose carry equals the global max (C0), pick the
             smallest t*FT + count[t]; gives the exact first-occurrence
             argmax index (counts are #strictly-below-carry per block)."""
    from concourse.dve_spec import Spec, Src0, Src1, C0, C1, C2, Idx, scan, AluOp

    maxscan = _register_dve_op(
        "ANT_MAX_SCAN_CARRY",
        Spec(body=scan(AluOp.MAX, Src0, init=C0), accum=AluOp.MAX),
    )
    # neg-candidate: (Src0-C0)*C2 - Src1, with Src1[t] = t*FT + cnt_t.  For
    # the block(s) whose carry Src0 equals the global max C0 this is
    # -(t*FT + cnt_t); for any other block Src0 < C0 and C2=7e10 pushes it
    # far below every real candidate (min fp32 gap * 7e10 > F).
    # accum MAX = -(first-occurrence argmax index).
    pick = _register_dve_op(
        "ANT_PICK_FIRST_MAX",
        Spec(body=(Src0 - C0) * C2 - Src1, accum=AluOp.MAX),
    )
    return maxscan, pick


def _build_nc(use_f32r=True, reps=1, stage='full'):
    import concourse.mybir as mybir
    import concourse.tile as tile
    import concourse.bass as bass
    from concourse import bacc

    f32 = mybir.dt.float32
    f32r = mybir.dt.float32r
    i32 = mybir.dt.int32
    Alu = mybir.AluOpType
    Act = mybir.ActivationFunctionType
    X = mybir.AxisListType.X

    nc = bacc.Bacc(None, target_bir_lowering=False)

    pred = nc.dram_tensor("pred", [NS, 3], f32, kind="ExternalInput")
    predT = nc.dram_tensor("predT", [3, NS], f32, kind="ExternalInput")
    opos = nc.dram_tensor("opos", [V, 3], f32, kind="ExternalInput")
    faces = nc.dram_tensor("faces", [F, 3], i32, kind="ExternalInput")
    out = nc.dram_tensor("out", [P, 2], f32, kind="ExternalOutput")
    tdram = nc.dram_tensor("tdram", [F, 4], f32, kind="Internal")
    caug = nc.dram_tensor("caug", [24, F], mybir.dt.bfloat16, kind="Internal")

    with tile.TileContext(nc) as tc:
        # ---------------- phase 0: gather face vertices + face table math ----------------
        # face tables (caug columns, tdram rows) are in natural face order
        # f = p*FP + i; phase-1 argmax indices use the same order.
        with tc.tile_pool(name="ph0", bufs=1) as ph0:
            faces_sb = ph0.tile([P, FP, 3], i32)
            nc.sync.dma_start(
                out=faces_sb[:], in_=faces[:].rearrange("(p i) k -> p i k", p=P)
            )
            fpl = ph0.tile([P, 3, FP], i32)
            for k in range(3):
                nc.vector.tensor_copy(out=fpl[:, k, :], in_=faces_sb[:, :, k])
            v = []
            for k in range(3):
                vk = ph0.tile([P, FP, 3], f32, name=f"v{k}")
                nc.gpsimd.indirect_dma_start(
                    out=vk[:],
                    out_offset=None,
                    in_=opos[:],
                    in_offset=bass.IndirectOffsetOnAxis(ap=fpl[:, k, :], axis=0),
                )
                v.append(vk)
            # centers
            cc = ph0.tile([P, FP, 3], f32)
            nc.vector.tensor_tensor(out=cc[:], in0=v[0][:], in1=v[1][:], op=Alu.add)
            nc.vector.tensor_tensor(out=cc[:], in0=cc[:], in1=v[2][:], op=Alu.add)
            nc.vector.tensor_scalar_mul(cc[:], cc[:], 1.0 / 3.0)
            # edges
            e1 = ph0.tile([P, FP, 3], f32)
            e2 = ph0.tile([P, FP, 3], f32)
            nc.vector.tensor_tensor(out=e1[:], in0=v[1][:], in1=v[0][:], op=Alu.subtract)
            nc.vector.tensor_tensor(out=e2[:], in0=v[2][:], in1=v[0][:], op=Alu.subtract)
            # cross product -> tint[:, :, 0:3]; plane offset b -> tint[:, :, 3]
            tint = ph0.tile([P, FP, 4], f32)
            tmp = ph0.tile([P, FP], f32)
            tmp2 = ph0.tile([P, FP], f32)
            for j in range(3):
                a, b2 = (j + 1) % 3, (j + 2) % 3
                nc.vector.tensor_tensor(out=tmp[:], in0=e1[:, :, a], in1=e2[:, :, b2], op=Alu.mult)
                nc.vector.tensor_tensor(out=tmp2[:], in0=e1[:, :, b2], in1=e2[:, :, a], op=Alu.mult)
                nc.vector.tensor_tensor(out=tint[:, :, j], in0=tmp[:], in1=tmp2[:], op=Alu.subtract)
            # norm
            nn2 = ph0.tile([P, FP], f32)
            nc.vector.tensor_tensor(out=nn2[:], in0=tint[:, :, 0], in1=tint[:, :, 0], op=Alu.mult)
            for j in (1, 2):
                nc.vector.tensor_tensor(out=tmp[:], in0=tint[:, :, j], in1=tint[:, :, j], op=Alu.mult)
                nc.vector.tensor_tensor(out=nn2[:], in0=nn2[:], in1=tmp[:], op=Alu.add)
            nc.scalar.sqrt(tmp[:], nn2[:])
            nc.vector.tensor_scalar_max(tmp[:], tmp[:], 1e-12)
            nc.vector.reciprocal(tmp2[:], tmp[:])
            for j in range(3):
                nc.vector.tensor_tensor(out=tint[:, :, j], in0=tint[:, :, j], in1=tmp2[:], op=Alu.mult)
            # b = c . n
            nc.vector.tensor_tensor(out=tmp[:], in0=cc[:, :, 0], in1=tint[:, :, 0], op=Alu.mult)
            nc.vector.tensor_tensor(out=tmp2[:], in0=cc[:, :, 1], in1=tint[:, :, 1], op=Alu.mult)
            nc.vector.tensor_tensor(out=tmp[:], in0=tmp[:], in1=tmp2[:], op=Alu.add)
            nc.vector.tensor_tensor(out=tmp2[:], in0=cc[:, :, 2], in1=tint[:, :, 2], op=Alu.mult)
            nc.vector.tensor_tensor(out=tint[:, :, 3], in0=tmp[:], in1=tmp2[:], op=Alu.add)
            # caug planar: rows c0,c1,c2,-|c|^2  stored [P, 4, FP]
            # (|c|^2 negated + paug p-rows at +2 => score' = 2p.c - |c|^2 = -score,
            #  so phase 1 runs an argMAX over score')
            cpl = ph0.tile([P, 4, FP], f32)
            for j in range(3):
                nc.vector.tensor_copy(out=cpl[:, j, :], in_=cc[:, :, j])
            nc.vector.tensor_tensor(out=cpl[:, 3, :], in0=cc[:, :, 0], in1=cc[:, :, 0], op=Alu.mult)
            for j in (1, 2):
                nc.vector.tensor_tensor(out=tmp[:], in0=cc[:, :, j], in1=cc[:, :, j], op=Alu.mult)
                nc.vector.tensor_tensor(out=cpl[:, 3, :], in0=cpl[:, 3, :], in1=tmp[:], op=Alu.add)
            nc.vector.tensor_scalar_mul(cpl[:, 3, :], cpl[:, 3, :], -1.0)
            bf = mybir.dt.bfloat16
            ch16 = ph0.tile([P, 4, FP], bf)
            nc.vector.tensor_copy(out=ch16[:], in_=cpl[:])
            chf = ph0.tile([P, 4, FP], f32)
            nc.vector.tensor_copy(out=chf[:], in_=ch16[:])
            cr1 = ph0.tile([P, 4, FP], f32)
            nc.vector.tensor_tensor(out=cr1[:], in0=cpl[:], in1=chf[:], op=Alu.subtract)
            cm16 = ph0.tile([P, 4, FP], bf)
            nc.vector.tensor_copy(out=cm16[:], in_=cr1[:])
            cmf = ph0.tile([P, 4, FP], f32)
            nc.vector.tensor_copy(out=cmf[:], in_=cm16[:])
            cr2 = ph0.tile([P, 4, FP], f32)
            nc.vector.tensor_tensor(out=cr2[:], in0=cr1[:], in1=cmf[:], op=Alu.subtract)
            cl16 = ph0.tile([P, 4, FP], bf)
            nc.vector.tensor_copy(out=cl16[:], in_=cr2[:])
            # rows: [bh, bm, bh, bl, bh, bm] paired with lhs [ah, ah, am, ah, al, am]
            cpl24 = ph0.tile([P, 24, FP], bf)
            for i, part in enumerate((ch16, cm16, ch16, cl16, ch16, cm16)):
                nc.vector.tensor_copy(out=cpl24[:, 4 * i:4 * i + 4, :], in_=part[:])
            nc.sync.dma_start(
                out=tdram[:].rearrange("(p i) k -> p i k", p=P), in_=tint[:]
            )
            nc.sync.dma_start(
                out=caug[:].rearrange("j (p i) -> p j i", p=P), in_=cpl24[:]
            )

        # ---------------- phase 1: scores + argmin ----------------
        bf = mybir.dt.bfloat16
        with tc.tile_pool(name="const1", bufs=1) as constp:
            paug = constp.tile([24, NS], bf)
            with tc.tile_pool(name="ptmp", bufs=1) as ptmp:
                paug0 = ptmp.tile([4, NS], f32)
                nc.vector.memset(paug0[:], 1.0)
                nc.sync.dma_start(out=paug0[0:3, :], in_=predT[:])
                nc.scalar.mul(paug0[0:3, :], paug0[0:3, :], 2.0)
                ph16 = ptmp.tile([4, NS], bf)
                nc.vector.tensor_copy(out=ph16[:], in_=paug0[:])
                phf = ptmp.tile([4, NS], f32)
                nc.vector.tensor_copy(out=phf[:], in_=ph16[:])
                pr1 = ptmp.tile([4, NS], f32)
                nc.vector.tensor_tensor(out=pr1[:], in0=paug0[:], in1=phf[:], op=Alu.subtract)
                pm16 = ptmp.tile([4, NS], bf)
                nc.vector.tensor_copy(out=pm16[:], in_=pr1[:])
                pmf = ptmp.tile([4, NS], f32)
                nc.vector.tensor_copy(out=pmf[:], in_=pm16[:])
                pr2 = ptmp.tile([4, NS], f32)
                nc.vector.tensor_tensor(out=pr2[:], in0=pr1[:], in1=pmf[:], op=Alu.subtract)
                pl16 = ptmp.tile([4, NS], bf)
                nc.vector.tensor_copy(out=pl16[:], in_=pr2[:])
                # lhs rows: [ah, ah, am, ah, al, am]
                for i, part in enumerate((ph16, ph16, pm16, ph16, pl16, pm16)):
                    nc.sync.dma_start(out=paug[4 * i:4 * i + 4, :], in_=part[:])
            maxscan, pickop = _get_ops()
            caug_sb = constp.tile([24, F], bf)
            nc.sync.dma_start(out=caug_sb[:], in_=caug[:])
            pred_pts = constp.tile([P, CHUNKS, 3], f32)
            nc.sync.dma_start(
                out=pred_pts[:], in_=pred[:].rearrange("(k p) j -> p k j", p=P)
            )
            idx_all = constp.tile([P, CHUNKS], i32)
            g = constp.tile([P, CHUNKS, 4], f32)
            iotaft = constp.tile([P, NBLK], f32)
            for t in range(NBLK):
                nc.vector.memset(iotaft[:, t:t + 1], float(t * FT))

            with (
                tc.tile_pool(name="psump", bufs=2, space="PSUM") as psump,
                tc.tile_pool(name="prefp", bufs=3) as prefp,
                tc.tile_pool(name="chkp", bufs=2) as chkp,
            ):
                for _rep in range(reps):
                  for k in range(CHUNKS):
                      lhsT = paug[:, k * P:(k + 1) * P]
                      carr = chkp.tile([P, NBLK], f32, tag="carr")
                      cnt = chkp.tile([P, NBLK], f32, tag="cnt")
                      for t in range(NBLK):
                          ps = psump.tile([P, FT], f32, tag="ps")
                          if stage not in ('nomm',):
                              for j in range(FT // 512):
                                  rr = caug_sb[:, t * FT + j * 512:t * FT + (j + 1) * 512]
                                  nc.tensor.matmul(
                                      ps[:, j * 512:(j + 1) * 512],
                                      lhsT,
                                      rr,
                                      start=True,
                                      stop=True,
                                  )
                          else:
                              nc.vector.memset(ps[:, 0:1], 0.0)
                          if stage not in ('noscan',):
                              prefix = prefp.tile([P, FT], f32, tag="prefix")
                              init = -BIG if t == 0 else carr[:, t - 1:t]
                              nc.vector._custom_dve(
                                  maxscan,
                                  out=prefix[:],
                                  in0=ps[:],
                                  s0=init,
                                  accum_out=carr[:, t:t + 1],
                              )
                              # cnt[t] = #(prefix < carry[t]) = argmax position
                              # within block when carry[t] is the global max
                              nc.scalar.activation(
                                  out=prefix[:],
                                  in_=prefix[:],
                                  func=Act.Sign,
                                  bias=carr[:, t:t + 1],
                                  scale=-1.0,
                                  accum_out=cnt[:, t:t + 1],
                              )
                      if stage in ('full', 'nogather'):
                          comb = chkp.tile([P, NBLK], f32, tag="comb")
                          nc.vector.tensor_tensor(out=comb[:], in0=cnt[:], in1=iotaft[:], op=Alu.add)
                          junk = chkp.tile([P, NBLK], f32, tag="junk")
                          idxf = chkp.tile([P, 1], f32, tag="idxf")
                          nc.vector._custom_dve(
                              pickop,
                              out=junk[:],
                              in0=carr[:],
                              in1=comb[:],
                              s0=carr[:, NBLK - 1:NBLK],
                              imm2=7e10,
                              accum_out=idxf[:],
                          )
                          nc.vector.tensor_scalar_mul(idx_all[:, k:k + 1], idxf[:], -1.0)
                      else:
                          nc.vector.memset(idx_all[:, k:k + 1], 0)
                      if stage != 'nogather':
                          nc.gpsimd.indirect_dma_start(
                              out=g[:, k, :],
                              out_offset=None,
                              in_=tdram[:],
                              in_offset=bass.IndirectOffsetOnAxis(ap=idx_all[:, k:k + 1], axis=0),
                          )

            # ---------------- finish ----------------
            with tc.tile_pool(name="finp", bufs=1) as finp:
                prod = finp.tile([P, CHUNKS, 3], f32)
                nc.vector.tensor_tensor(out=prod[:], in0=g[:, :, 0:3], in1=pred_pts[:], op=Alu.mult)
                s3 = finp.tile([P, CHUNKS], f32)
                nc.vector.tensor_reduce(out=s3[:], in_=prod[:], axis=X, op=Alu.add)
                d = finp.tile([P, CHUNKS], f32)
                nc.vector.tensor_tensor(out=d[:], in0=s3[:], in1=g[:, :, 3], op=Alu.subtract)
                interp = finp.tile([P, CHUNKS], f32)
                eps1 = finp.tile([P, 1], f32)
                nc.vector.memset(eps1[:], EPS)
                nc.scalar.activation(out=interp[:], in_=d[:], func=Act.Relu, bias=eps1[:, 0:1], scale=-1.0)
                outsb = finp.tile([P, 2], f32)
                sgn = finp.tile([P, CHUNKS], f32)
                nc.scalar.activation(
                    out=sgn[:], in_=interp[:], func=Act.Sign, bias=0.0, scale=1.0,
                    accum_out=outsb[:, 1:2],
                )
                sq = finp.tile([P, CHUNKS], f32)
                nc.scalar.square(sq[:], interp[:])
                cube = finp.tile([P, CHUNKS], f32)
                nc.vector.tensor_tensor(out=cube[:], in0=sq[:], in1=interp[:], op=Alu.mult)
                nc.vector.tensor_reduce(out=outsb[:, 0:1], in_=cube[:], axis=X, op=Alu.add)
                nc.sync.dma_start(out=out[:], in_=outsb[:])

    nc.compile()
    return nc


def _get_nc():
    if "nc" not in _CACHE:
        _CACHE["nc"] = _build_nc()
    return _CACHE["nc"]


def _make_in_maps(pred_pos, obstacle_pos, obstacle_faces):
    pred_pos = np.ascontiguousarray(np.asarray(pred_pos, dtype=np.float32))
    obstacle_pos = np.ascontiguousarray(np.asarray(obstacle_pos, dtype=np.float32))
    faces = np.ascontiguousarray(np.asarray(obstacle_faces).astype(np.int32))
    in_maps = []
    for c in range(NCORES):
        b, half = c // SPLIT, c % SPLIT
        pr = np.ascontiguousarray(pred_pos[b, half * NS:(half + 1) * NS])
        in_maps.append({
            "pred": pr,
            "predT": np.ascontiguousarray(pr.T),
            "opos": obstacle_pos[b],
            "faces": faces[b],
        })
    return in_maps


def kernel(pred_pos, obstacle_pos, obstacle_faces):
    from concourse.bass_utils import run_bass_kernel_spmd

    nc = _get_nc()
    in_maps = _make_in_maps(pred_pos, obstacle_pos, obstacle_faces)
    res = run_bass_kernel_spmd(nc, in_maps, core_ids=list(range(NCORES)))
    outs = np.stack([r["out"] for r in res.results])  # [8, 128, 2]
    loss_sum = float(outs[:, :, 0].astype(np.float64).sum())
    cnt_sum = float(outs[:, :, 1].astype(np.float64).sum())
    loss = np.float32(loss_sum / B * WEIGHT)
    perc = np.float32(cnt_sum / (B * N))
    return loss, perc

